# revision 54
# baseline (speedup 1.0000x reference)
"""BatchedGCN Trainium2 kernel (optimized).

Per graph (batch element):
  norms_i = ||X_i||;  A = (cos_sim > 0.3) + I ; deg = rowsum(A); d = deg^-1/2
  H1 = relu(diag(d) A diag(d) (X @ W1.T) + b1)
  H2 = diag(d) A diag(d) (H1 @ W2.T) + b2
  out = H2 / max(||H2_row||, 1e-12)

Design notes (per core: 4 graphs, weights replicated; B=32 over 8 cores):
- X ships twice: fp8 DoubleRow pair-interleaved X8 ([k, p, i, n],
  d = k*256+i*128+p) for the gram matrix, bf16 X^T for X@W1.T.  All X8
  loads go first on one queue; X^T loads follow on the same queue so the
  latency-critical gram inputs get full fabric bandwidth.
- Row norms come from the gram diagonal blocks (fp8 DR matmuls); the
  threshold comparison runs un-normalized as (G * 1/n_i > t*n_j), so only
  the bound needs norms.  t*n_j is replicated across partitions entirely
  on-chip: PE-transpose [128,8] -> [8,128], then K=8 row-selector matmuls
  (eyerows) broadcast each row -- no DRAM bounce on the critical path.
- The threshold (DVE STT, deg fused via accum) writes A straight into
  fp8 DR-packed tiles [jd, i, n]; A entries {0,1,2} are exact in fp8.
- Both propagations run as fp8 DoubleRow matmuls with compensated pairs
  Ys ~= y8 + r8 (residual also fp8, values pre-scaled x16 to clear the
  fp8 subnormal floor): half the bf16 streaming cost at ~0.1% error.
- prop2 accumulates H2^T ([dout, n]) in wide 512-col DR matmuls; PE
  transposes bring H2 tiles to [n, dout] in PSUM, the scalar engine
  copies each tile to SBUF, and a DVE STT (PSUM x SBUF copy) squares it
  with accum_out -- per-node ssq lands directly in partition layout, no
  ones-matmul / replicated-row transpose dance.  The 1/norm scale is a
  DVE tensor_scalar over the SBUF copy.
- DRAM layouts are partition-major ([b, p, ...]) so every DMA moves
  2-12KB contiguous per partition; the host re-packs X and un-packs Y.
- d^-1/2 chains, biases, and weight columns avoid partition-stride-1
  DMAs (4-byte-descriptor grinds); biases load as rows + PE transpose.
- Emission is wave-pipelined and engine-balanced: diag(g)/nrep(g-1)
  interleave, gram(g)/xw1(g-1) interleave, phase_e fills tensor gaps in
  the per-half phase_f pipeline (f2 trails f1 by two halves).  The
  engine balance is delicate: the f-region paces on scalar+DVE jointly
  and the gram region on DVE thresholds -- moving ops between engines or
  reordering phases was measured slower every time (HAM re-throttle on
  PE idle gaps; see git-less history in the session notes).
- The PE clock is power-state dependent (2.4 vs 2.0 GHz) -- compare
  timings only at equal DR-512 matmul gaps (216ns = 2.4GHz).
"""
from contextlib import ExitStack

import ml_dtypes
import numpy as np

import concourse.bass as bass
import concourse.mybir as mybir
import concourse.tile as tile
from concourse import bacc
from concourse.bass_utils import run_bass_kernel_spmd
from concourse.masks import make_identity

B, N, D_IN, D_H, D_OUT = 32, 1024, 768, 256, 128
N_CORES = 8
BPC = B // N_CORES          # graphs per core
NT = N // 128               # 8 node row tiles
KDR = D_IN // 256           # 3 DoubleRow K-chunks over D_in
NJD = N // 256              # 4 DoubleRow K-chunks over nodes
DTI = D_IN // 128           # 6 input-dim tiles
HC = D_H // 128             # 2 hidden chunks
F32 = mybir.dt.float32
BF16 = mybir.dt.bfloat16
FP8 = mybir.dt.float8e4

KNN_THRESHOLD = 0.3
COS_EPS = 1e-8
NORM_EPS = 1e-12
ALU = mybir.AluOpType
AF = mybir.ActivationFunctionType
DR = mybir.MatmulPerfMode.DoubleRow
Y_SCALE = 16.0              # pre-scale of fp8 Ys pairs (subnormal avoidance)


def build(n_batches: int = BPC):
    nc = bacc.Bacc("TRN2", debug=False, num_devices=N_CORES)
    # partition-major DRAM layouts: per-partition contiguous runs give
    # 2-12KB DMA elements instead of 512B-2KB descriptors
    X8 = nc.dram_tensor("X8", [n_batches, 128, KDR, 2, N], FP8,
                        kind="ExternalInput")
    XT = nc.dram_tensor("XT", [n_batches, 128, DTI, N], BF16,
                        kind="ExternalInput")
    W1T = nc.dram_tensor("W1T", [D_IN, D_H], BF16, kind="ExternalInput")
    W2T = nc.dram_tensor("W2T", [D_H, D_OUT], BF16, kind="ExternalInput")
    b1 = nc.dram_tensor("b1", [D_H], F32, kind="ExternalInput")
    b2 = nc.dram_tensor("b2", [D_OUT], F32, kind="ExternalInput")
    Y = nc.dram_tensor("Y", [n_batches, 128, NT, D_OUT], F32,
                       kind="ExternalOutput")
    with tile.TileContext(nc) as tc, ExitStack() as ctx:
        _body(ctx, tc, X8.ap(), XT.ap(), W1T.ap(), W2T.ap(),
              b1.ap(), b2.ap(), Y.ap(), n_batches)
    nc.compile()
    return nc


def _bcast_p(ap: bass.AP, parts: int = 128) -> bass.AP:
    """Broadcast a DRAM AP across `parts` partitions (partition-stride 0)."""
    return bass.AP(tensor=ap.tensor, offset=ap.offset, ap=[[0, parts]] + list(ap.ap))


class _G:
    """Per-graph state threaded between pipeline phases."""
    __slots__ = ("X8b", "XTb", "Yb", "x8", "xt", "at", "y8", "ry", "y2", "r2",
                 "h1t", "ssqv", "nct", "rc", "nrep", "degv", "dv", "dvw",
                 "dvy", "dvb", "drep", "h2tb", "ssqn")


def _body(ctx, tc, X8, XT, W1T, W2T, b1, b2, Y, n_batches):
    nc = tc.nc
    nb = n_batches

    singles = ctx.enter_context(tc.tile_pool(name="singles", bufs=1))
    xpool = ctx.enter_context(tc.tile_pool(name="xpool", bufs=nb))
    apool = ctx.enter_context(tc.tile_pool(name="apool", bufs=nb))
    ypool = ctx.enter_context(tc.tile_pool(name="ypool", bufs=nb))
    h1pool = ctx.enter_context(tc.tile_pool(name="h1pool", bufs=nb * HC))
    rppool = ctx.enter_context(tc.tile_pool(name="rppool", bufs=nb))
    bvec = ctx.enter_context(tc.tile_pool(name="bvec", bufs=nb))
    sqj = ctx.enter_context(tc.tile_pool(name="sqj", bufs=2))
    tmppool = ctx.enter_context(tc.tile_pool(name="tmppool", bufs=4))
    h2pool = ctx.enter_context(tc.tile_pool(name="h2pool", bufs=2))
    opool = ctx.enter_context(tc.tile_pool(name="opool", bufs=2))
    psA = ctx.enter_context(tc.tile_pool(name="psA", bufs=4, space="PSUM"))
    psB = ctx.enter_context(tc.tile_pool(name="psB", bufs=4, space="PSUM"))
    dramp = ctx.enter_context(tc.tile_pool(name="dramp", bufs=nb, space="DRAM"))

    # ---- one-time constants ------------------------------------------------
    ident = singles.tile([128, 128], BF16)
    make_identity(nc, ident)
    identf = singles.tile([128, 128], F32)
    make_identity(nc, identf)
    ident2 = singles.tile([128, 2, 128], FP8)
    nc.gpsimd.memset(ident2, 0.0)
    make_identity(nc, ident2[:, 0, :], nomemset=True)
    make_identity(nc, ident2[:, 1, :], nomemset=True)
    # eyerows[k, it, :] == 1 iff k == it: K=8 row-selector for broadcasts
    eyerows = singles.tile([NT, NT, 128], BF16)
    nc.gpsimd.memset(eyerows, 0.0)
    nc.gpsimd.affine_select(out=eyerows, in_=eyerows,
                            compare_op=mybir.AluOpType.not_equal, fill=1.0,
                            base=0, pattern=[[-1, NT], [0, 128]],
                            channel_multiplier=1)
    ceps = singles.tile([128, 1], F32)
    nc.gpsimd.memset(ceps, COS_EPS * COS_EPS)
    cneps = singles.tile([128, 1], F32)
    nc.gpsimd.memset(cneps, NORM_EPS * NORM_EPS)

    b1col = singles.tile([128, HC], F32)
    b2col = singles.tile([128, 1], F32)
    b1row = singles.tile([HC, 128], F32)
    b2row = singles.tile([1, 128], F32)
    w1t = singles.tile([128, DTI, D_H], BF16)
    w2t = [singles.tile([128, D_OUT], BF16, tag=f"w2t{k}", name=f"w2t{k}")
           for k in range(HC)]
    # warm the scalar activation table before any loads land
    warmt = singles.tile([128, 1], F32)
    nc.scalar.activation(out=warmt, in_=ceps, func=AF.Sqrt, bias=1.0)

    def load_weights():
        # emitted after the per-graph X loads so those win the DMA queues;
        # biases load as contiguous rows and PE-transpose to column form
        # (a partition-stride-1 load is a 4-byte-descriptor grind)
        nc.scalar.dma_start(out=b1row, in_=bass.AP(
            tensor=b1.tensor, offset=b1.offset, ap=[[128, HC], [1, 128]]))
        nc.scalar.dma_start(out=b2row, in_=bass.AP(
            tensor=b2.tensor, offset=b2.offset, ap=[[128, 1], [1, 128]]))
        psb = psB.tile([128, 128], F32, tag="psB", name="psb")
        nc.tensor.transpose(psb[:, 0:HC], b1row, identf[:HC, :HC])
        nc.scalar.copy(out=b1col, in_=psb[:, 0:HC])
        psb2 = psB.tile([128, 128], F32, tag="psB", name="psb2")
        nc.tensor.transpose(psb2[:, 0:1], b2row, identf[:1, :1])
        nc.scalar.copy(out=b2col, in_=psb2[:, 0:1])
        nc.scalar.dma_start(out=w1t, in_=bass.AP(
            tensor=W1T.tensor, offset=W1T.offset,
            ap=[[D_H, 128], [128 * D_H, DTI], [1, D_H]]))
        for k in range(HC):
            nc.scalar.dma_start(out=w2t[k], in_=W2T[k * 128:(k + 1) * 128, :])

    t2 = KNN_THRESHOLD * KNN_THRESHOLD

    # ---- per-phase emitters ------------------------------------------------
    def phase_load(g: _G):
        # per-k chunk loads so the first norm matmuls start as soon as the
        # first 256KB chunk lands (instead of the full 768KB)
        g.x8 = xpool.tile([128, KDR, 2, N], FP8, tag="x8")
        for k in range(KDR):
            nc.sync.dma_start(out=g.x8[:, k], in_=bass.AP(
                tensor=g.X8b.tensor, offset=g.X8b.offset + k * 2 * N,
                ap=[[KDR * 2 * N, 128], [N, 2], [1, N]]))
        g.xt = xpool.tile([128, DTI, N], BF16, tag="xt", bufs=nb)

    def load_xt(g: _G):
        # same queue as the x8 loads, issued after all of them: FIFO order
        # keeps the fp8 gram inputs at full fabric bandwidth
        nc.sync.dma_start(out=g.xt, in_=bass.AP(
            tensor=g.XTb.tensor, offset=g.XTb.offset,
            ap=[[DTI * N, 128], [N, DTI], [1, N]]))

    def phase_norm(g: _G):
        # row norms from gram diagonal blocks; produce t*n_j (bounced to a
        # partition-replicated row) and rc_i = 1/n_i (per-partition scalars).
        # k-outer so the k=0 sweep runs while chunks 1-2 are still in flight.
        g.ssqv = bvec.tile([128, NT], F32, tag="ssqv")
        psd = [psB.tile([128, 512], F32, tag="psB", name=f"psd{h}")
               for h in range(2)]
        for k in range(KDR):
            for it in range(NT):
                blk = slice(it * 128, (it + 1) * 128)
                nc.tensor.matmul(psd[it // 4][:, (it % 4) * 128:(it % 4 + 1) * 128],
                                 lhsT=g.x8[:, k, :, blk],
                                 rhs=g.x8[:, k, :, blk],
                                 start=(k == 0), stop=(k == KDR - 1),
                                 perf_mode=DR)
        for it in range(NT):
            dj = sqj.tile([128, 128], BF16, tag="dj")
            nc.vector.scalar_tensor_tensor(
                out=dj, in0=psd[it // 4][:, (it % 4) * 128:(it % 4 + 1) * 128],
                scalar=1.0, in1=identf,
                op0=ALU.bypass, op1=ALU.mult,
                accum_out=g.ssqv[:, it:it + 1])
        g.nct = bvec.tile([128, NT], BF16, tag="nct")
        nc.scalar.activation(out=g.nct, in_=g.ssqv, func=AF.Sqrt, scale=t2)
        nclp = bvec.tile([128, NT], F32, tag="nclp")
        nc.scalar.activation(out=nclp, in_=g.ssqv, func=AF.Sqrt, bias=ceps)
        g.rc = bvec.tile([128, NT], F32, tag="rc")
        nc.vector.reciprocal(out=g.rc, in_=nclp)
    def phase_nrep(g: _G):
        # transpose [128, NT] -> [NT, 128] so the DRAM spill is 8 contiguous
        # rows (a partition-major spill would be a 4-byte-descriptor grind);
        # emitted a graph late so the transpose never stalls the PE stream
        psn = psB.tile([128, 128], BF16, tag="psB", name="psn")
        nc.tensor.transpose(psn[:NT, :], g.nct, ident)
        nctT = sqj.tile([NT, 128], BF16, tag="nctT")
        nc.scalar.copy(out=nctT, in_=psn[:NT, :])
        # replicate row it of nctT across all partitions with K=1 matmuls:
        # no DRAM round trip on the startup critical path
        g.nrep = rppool.tile([128, N], BF16, tag="nrep")
        for nh in range(2):
            psr = psB.tile([128, 512], F32, tag="psB", name="psr")
            for itl in range(4):
                it = nh * 4 + itl
                nc.tensor.matmul(psr[:, itl * 128:(itl + 1) * 128],
                                 lhsT=eyerows[:, it, :], rhs=nctT,
                                 start=True, stop=True)
            nc.scalar.copy(out=g.nrep[:, nh * 512:(nh + 1) * 512], in_=psr)

    def phase_gram(g: _G):
        # G row tiles -> threshold -> A in fp8 DR-packed tiles, deg fused
        g.degv = bvec.tile([128, 2 * NT], F32, tag="degv")
        for it in range(NT):
            jd, i = it // 2, it % 2
            blk = slice(it * 128, (it + 1) * 128)
            for jh in range(2):
                ps = psA.tile([128, 512], F32, tag="psA")
                for k in range(KDR):
                    nc.tensor.matmul(
                        ps, lhsT=g.x8[:, k, :, blk],
                        rhs=g.x8[:, k, :, jh * 512:(jh + 1) * 512],
                        start=(k == 0), stop=(k == KDR - 1), perf_mode=DR)
                nc.vector.scalar_tensor_tensor(
                    out=g.at[jd][:, i, jh * 512:(jh + 1) * 512], in0=ps,
                    scalar=g.rc[:, it:it + 1],
                    in1=g.nrep[:, jh * 512:(jh + 1) * 512],
                    op0=ALU.mult, op1=ALU.is_gt,
                    accum_out=g.degv[:, jh * NT + it:jh * NT + it + 1])
        for jd in range(NJD):
            # self loops: add I to both diag blocks of the DR pair in one op
            sl = g.at[jd][:, 0, 2 * jd * 128:2 * jd * 128 + 128]
            dview = bass.AP(tensor=sl.tensor, offset=sl.offset,
                            ap=[list(sl.ap[0]), [N + 128, 2], [1, 128]])
            nc.gpsimd.tensor_add(out=dview, in0=dview, in1=ident2)

        dsum = bvec.tile([128, NT], F32, tag="dsum")
        nc.vector.tensor_tensor(out=dsum, in0=g.degv[:, 0:NT],
                                in1=g.degv[:, NT:2 * NT], op=ALU.add)
        sqd = bvec.tile([128, NT], F32, tag="sqd")
        nc.scalar.activation(out=sqd, in_=dsum, func=AF.Sqrt, bias=1.0)
        g.dv = bvec.tile([128, NT], F32, tag="dv")
        nc.vector.reciprocal(out=g.dv, in_=sqd)
        g.dvw = bvec.tile([128, NT], F32, tag="dvw")
        nc.vector.tensor_scalar_mul(g.dvw, g.dv, Y_SCALE)
        g.dvy = bvec.tile([128, NT], F32, tag="dvy")
        nc.vector.tensor_scalar_mul(g.dvy, g.dv, Y_SCALE)
        g.dvb = bvec.tile([128, NT], BF16, tag="dvb")
        nc.vector.tensor_scalar_mul(g.dvb, g.dv, 1.0 / Y_SCALE)

    def phase_drep(g: _G):
        # bounce d/Y_SCALE to a partition-replicated row (transposed spill,
        # emitted after xw1 so the tensor engine never waits on the chain)
        psv = psB.tile([128, 128], BF16, tag="psB", name="psv")
        nc.tensor.transpose(psv[:NT, :], g.dvb, ident)
        dvT = sqj.tile([NT, 128], BF16, tag="dvT")
        nc.scalar.copy(out=dvT, in_=psv[:NT, :])
        dscr = dramp.tile([1, N], BF16, tag="dscr")
        dflat = dscr[0]
        nc.gpsimd.dma_start(
            out=bass.AP(tensor=dflat.tensor, offset=dflat.offset,
                        ap=[[128, NT], [1, 128]]),
            in_=dvT)
        g.drep = rppool.tile([128, N], BF16, tag="drep")
        nc.gpsimd.dma_start(out=g.drep, in_=_bcast_p(dflat))

    def phase_xw1(g: _G):
        # G1 = X @ W1.T via compensated fp8: X8@W18 + X8@W1r8 + R8@W18;
        # evict d_j-scaled as fp8 pair (y8, ry) for the DR propagation.
        for it in range(NT):
            jd, i = it // 2, it % 2
            blk = slice(it * 128, (it + 1) * 128)
            ps = psB.tile([128, D_H], F32, tag="psB")
            for dt in range(DTI):
                nc.tensor.matmul(ps, lhsT=g.xt[:, dt, blk], rhs=w1t[:, dt],
                                 start=(dt == 0), stop=(dt == DTI - 1))
            y8sl = g.y8[:, jd, i, :]
            nc.scalar.activation(out=y8sl, in_=ps, func=AF.Copy,
                                 scale=g.dvw[:, it:it + 1])
            nc.vector.scalar_tensor_tensor(
                out=g.ry[:, jd, i, :], in0=ps, scalar=g.dvw[:, it:it + 1],
                in1=y8sl, op0=ALU.mult, op1=ALU.subtract)

    def phase_prop1(g: _G):
        # M1^T = (A diag(d) G1)^T via DR pairs; H1^T = relu(d_i * M1^T + b1)
        pss = {}
        for hc in range(HC):
            g.h1t.append(h1pool.tile([128, N], BF16, tag="h1", name="h1"))
            for ih in range(2):
                pss[hc, ih] = psA.tile([128, 512], F32, tag="psA", name="psd2")
        nsrc = 2 * NJD
        step = 0
        for jd in range(NJD):
            for src in (g.y8, g.ry):
                st = step == 0
                sp = step == nsrc - 1
                step += 1
                for hc in range(HC):
                    lhsT = src[:, jd, :, hc * 128:(hc + 1) * 128]
                    for ih in range(2):
                        nc.tensor.matmul(
                            pss[hc, ih], lhsT=lhsT,
                            rhs=g.at[jd][:, :, ih * 512:(ih + 1) * 512],
                            start=st, stop=sp, perf_mode=DR)
        for hc in range(HC):
            for ih in range(2):
                tmp = tmppool.tile([128, 512], F32, tag="tmp")
                nc.vector.tensor_tensor(out=tmp, in0=pss[hc, ih],
                                        in1=g.drep[:, ih * 512:(ih + 1) * 512],
                                        op=ALU.mult)
                nc.scalar.activation(out=g.h1t[hc][:, ih * 512:(ih + 1) * 512],
                                     in_=tmp, func=AF.Relu,
                                     bias=b1col[:, hc:hc + 1])

    def phase_e(g: _G, its=range(NT)):
        # Ys2 = d_j * (H1 @ W2.T), evicted as fp8 pair (y2, r2)
        for it in its:
            jd, i = it // 2, it % 2
            ps = psB.tile([128, D_OUT], F32, tag="psB", name="psE")
            for hc in range(HC):
                nc.tensor.matmul(ps, lhsT=g.h1t[hc][:, it * 128:(it + 1) * 128],
                                 rhs=w2t[hc], start=(hc == 0),
                                 stop=(hc == HC - 1))
            y2sl = g.y2[:, jd, i, :]
            nc.scalar.activation(out=y2sl, in_=ps, func=AF.Copy,
                                 scale=g.dvy[:, it:it + 1])
            nc.vector.scalar_tensor_tensor(
                out=g.r2[:, jd, i, :], in0=ps, scalar=g.dvy[:, it:it + 1],
                in1=y2sl, op0=ALU.mult, op1=ALU.subtract)

    def phase_f1(g: _G, ih: int):
        # H2^T half = (A Ys2)^T in wide DR matmuls; b2 add rides the scalar
        # engine (Copy with per-partition bias)
        if ih == 0:
            g.h2tb = h2pool.tile([128, N], BF16, tag="h2tb", bufs=nb)
            g.ssqn = bvec.tile([128, NT], F32, tag="ssqn")
        nsrc = 2 * NJD
        ps2 = psA.tile([128, 512], F32, tag="psA", name="ps2")
        step = 0
        for jd in range(NJD):
            for src in (g.y2, g.r2):
                nc.tensor.matmul(
                    ps2, lhsT=src[:, jd],
                    rhs=g.at[jd][:, :, ih * 512:(ih + 1) * 512],
                    start=(step == 0), stop=(step == nsrc - 1),
                    perf_mode=DR)
                step += 1
        half = slice(ih * 512, (ih + 1) * 512)
        tmp = tmppool.tile([128, 512], F32, tag="tmp")
        nc.vector.tensor_tensor(out=tmp, in0=ps2, in1=g.drep[:, half],
                                op=ALU.mult)
        nc.scalar.activation(out=g.h2tb[:, half], in_=tmp, func=AF.Identity,
                             bias=b2col)

    def phase_f2(g: _G, ih: int):
        # PE transposes to [n, dout]; scalar copies each tile to SBUF, DVE
        # squares (PSUM x SBUF) with accum giving per-node ssq directly --
        # psf's PSUM bank frees right after the squares, and the 1/norm
        # scale reads the SBUF copy
        obuf = opool.tile([128, 512], F32, tag="obuf")
        psf = psB.tile([128, 512], BF16, tag="psB", name="psf")
        obr = sqj.tile([128, 512], BF16, tag="obr", bufs=4)
        sqscr = sqj.tile([128, 512], BF16, tag="sq")
        for itl in range(4):
            it = ih * 4 + itl
            sl = slice(itl * 128, (itl + 1) * 128)
            nc.tensor.transpose(psf[:, sl], g.h2tb[:, it * 128:(it + 1) * 128],
                                ident)
            nc.scalar.copy(out=obr[:, sl], in_=psf[:, sl])
            nc.vector.scalar_tensor_tensor(
                out=sqscr[:, sl], in0=psf[:, sl], scalar=1.0, in1=obr[:, sl],
                op0=ALU.bypass, op1=ALU.mult,
                accum_out=g.ssqn[:, it:it + 1])
        srl = bvec.tile([128, NT // 2], F32, tag="srl")
        nc.scalar.activation(out=srl, in_=g.ssqn[:, ih * 4:(ih + 1) * 4],
                             func=AF.Sqrt, bias=cneps)
        rli = bvec.tile([128, NT // 2], F32, tag="rli")
        nc.vector.reciprocal(out=rli, in_=srl)
        for itl in range(4):
            nc.vector.tensor_scalar(out=obuf[:, itl * 128:(itl + 1) * 128],
                                    in0=obr[:, itl * 128:(itl + 1) * 128],
                                    scalar1=rli[:, itl:itl + 1],
                                    scalar2=None, op0=ALU.mult)
        nc.sync.dma_start(
            out=bass.AP(tensor=g.Yb.tensor,
                        offset=g.Yb.offset + ih * 4 * D_OUT,
                        ap=[[NT * D_OUT, 128], [D_OUT, NT // 2], [1, D_OUT]]),
            in_=obuf)

    # ---- wave-pipelined driver ---------------------------------------------
    gs = []
    for bi in range(nb):
        g = _G()
        g.X8b, g.XTb, g.Yb = X8[bi], XT[bi], Y[bi]
        g.h1t = []
        g.at = []
        gs.append(g)

    load_weights()
    for g in gs:
        phase_load(g)
    for g in gs:
        load_xt(g)
    for g in gs:
        # A tiles allocated up front so the threshold can write DR slices
        for jd in range(NJD):
            g.at.append(apool.tile([128, 2, N], FP8, tag="at", bufs=nb * NJD,
                                   name="at"))
        g.y8 = ypool.tile([128, NJD, 2, D_H], FP8, tag="y8")
        g.ry = ypool.tile([128, NJD, 2, D_H], FP8, tag="ry", bufs=nb)
        g.y2 = ypool.tile([128, NJD, 2, D_OUT], FP8, tag="y2", bufs=nb)
        g.r2 = ypool.tile([128, NJD, 2, D_OUT], FP8, tag="r2", bufs=nb)
    phase_norm(gs[0])
    for gi in range(1, nb):
        phase_norm(gs[gi])
        phase_nrep(gs[gi - 1])
    phase_nrep(gs[nb - 1])
    # xw1(g-1) is emitted between gram(g-1) and gram(g) so its DVE residual
    # evictions drain while the tensor engine streams the next graph's gram
    phase_gram(gs[0])
    for gi in range(1, nb):
        phase_xw1(gs[gi - 1])
        phase_drep(gs[gi - 1])
        phase_gram(gs[gi])
    phase_xw1(gs[nb - 1])
    phase_drep(gs[nb - 1])
    for g in gs:
        phase_prop1(g)
    # phase_e emissions are interleaved into the f pipeline as tensor-engine
    # filler while each half's normalize chain drains on vector/scalar/gpsimd
    phase_e(gs[0])
    phase_e(gs[1])
    halves = [(g, ih) for g in gs for ih in range(2)]
    nh = len(halves)
    for i in range(nh):
        phase_f1(*halves[i])
        # e(g) split into two half-graph blocks on consecutive halves:
        # spreads the eviction load on scalar/DVE instead of bunching it
        if 1 <= i <= 2 * (nb - 2):
            eg = 2 + (i - 1) // 2
            half4 = ((i - 1) % 2) * 4
            phase_e(gs[eg], its=range(half4, half4 + 4))
        if i >= 2:
            phase_f2(*halves[i - 2])
    phase_f2(*halves[nh - 2])
    phase_f2(*halves[nh - 1])


_NC_CACHE = {}


def _get_nc(n_batches: int = BPC):
    if n_batches not in _NC_CACHE:
        _NC_CACHE[n_batches] = build(n_batches)
    return _NC_CACHE[n_batches]


def _dr_pack(a: np.ndarray) -> np.ndarray:
    """[b, D, N] -> partition-major DoubleRow pair-interleave [b, p, k, i, n]."""
    nb = a.shape[0]
    return np.ascontiguousarray(
        a.reshape(nb, KDR, 2, 128, N).transpose(0, 3, 1, 2, 4))


def make_in_maps(X, W1, b1, W2, b2, bpc: int = BPC):
    X = np.asarray(X, dtype=np.float32)
    nb = len(X)
    Xt = np.ascontiguousarray(X.transpose(0, 2, 1))        # [B, D, N] f32
    X8 = _dr_pack(Xt.astype(ml_dtypes.float8_e4m3))
    XT16 = np.ascontiguousarray(                           # [B, 128, DTI, N]
        Xt.astype(ml_dtypes.bfloat16).reshape(nb, DTI, 128, N)
        .transpose(0, 2, 1, 3))
    W1T = np.ascontiguousarray(
        np.asarray(W1, dtype=np.float32).T.astype(ml_dtypes.bfloat16))
    W2T = np.ascontiguousarray(
        np.asarray(W2, dtype=np.float32).T.astype(ml_dtypes.bfloat16))
    b1 = np.ascontiguousarray(np.asarray(b1, dtype=np.float32))
    b2 = np.ascontiguousarray(np.asarray(b2, dtype=np.float32))
    return [
        {"X8": X8[c * bpc:(c + 1) * bpc], "XT": XT16[c * bpc:(c + 1) * bpc],
         "W1T": W1T, "W2T": W2T, "b1": b1, "b2": b2}
        for c in range(nb // bpc)
    ]


def unpack_y(results):
    yp = np.concatenate([r["Y"] for r in results], axis=0)
    # [B, 128, NT, D_OUT] partition-major -> [B, N, D_OUT] (n = it*128 + p)
    return np.ascontiguousarray(yp.transpose(0, 2, 1, 3).reshape(-1, N, D_OUT))


def kernel(X, W1, b1, W2, b2):
    nc = _get_nc()
    in_maps = make_in_maps(X, W1, b1, W2, b2)
    res = run_bass_kernel_spmd(nc, in_maps, core_ids=list(range(N_CORES)))
    return unpack_y(res.results)

